# revision 12
# baseline (speedup 1.0000x reference)
"""Trainium2 Bass kernel for nn_MultiHeadAttention_66872640799208.

Math (per batch element b, S=2048, D=1024):
    qp = q @ Wq.T + bq ; kp = k @ Wk.T + bk ; vp = v @ Wv.T + bv
    scores = qp @ kp.T / D
    probs  = softmax(scores, axis=q)          # over the QUERY axis
    attn   = probs @ vp
    attn_w = softmax(attn, axis=q)            # over the sequence axis
    out    = (attn + q, attn_w)

Sharding: data-parallel over batch B=8 -> one batch element per NeuronCore,
no collectives. Host pre-transposes inputs to bf16 so that every matmul
contracts over the partition axis with no on-chip transposes:
  - qT/kT/vT [D, S] feed the projections (contraction over d),
  - qpT/kpT are produced in [e, s] layout so scoresT = kpT.T @ qpT has the
    softmax axis (q) on the free dimension,
  - vp is produced in natural [s, e] layout as lhsT/rhs of the attn matmul,
  - the probs 1/Z normalization (Z indexed by k) is folded into vp's rows
    (k is vp's partition axis) instead of scaling the much larger probs.
The second softmax (over q, the partition axis of attn) uses a ones-vector
PE matmul for the column sums and a K=1 PE matmul to broadcast 1/Z.

SBUF (192KB/partition budget) is managed with tag-slot reuse in one pool:
  tag A: qp (P1-P2) -> expb=exp(attn) (P3-P4)        32KB
  tag B: kp (P1-P2)                                  32KB
  tag W: weights (P1) -> probs (P2-P3)               64KB (max)
  tag V: vp (P1-P3)                                  32KB
"""

import sys

if "/opt/trn_rl_repo" not in sys.path:
    sys.path.insert(0, "/opt/trn_rl_repo")

import numpy as np
import ml_dtypes

B = 8
S = 2048
D = 1024
P = 128


def build_nc(s=S, d=D):
    """Build the single-core Bass program (SPMD: identical on all cores)."""
    import concourse.bass as bass
    import concourse.tile as tile
    from concourse import bacc, mybir

    bf16 = mybir.dt.bfloat16
    f32 = mybir.dt.float32

    DT = d // P          # contraction tiles for projections
    ET = d // P          # e (feature) tiles
    ST = s // P          # sequence tiles
    NFQ = min(512, s)    # matmul moving free-dim over q
    QC = s // NFQ        # q chunks
    NFD = min(512, d)    # matmul moving free-dim over d/e
    EC = d // NFD        # e chunks
    SCW = min(512, s)    # vT stream chunk width (in s)
    SC = s // SCW

    nc = bacc.Bacc("TRN2")

    qT = nc.dram_tensor("qT", [d, s], bf16, kind="ExternalInput")
    kT = nc.dram_tensor("kT", [d, s], bf16, kind="ExternalInput")
    vT = nc.dram_tensor("vT", [d, s], bf16, kind="ExternalInput")
    wqT = nc.dram_tensor("wqT", [d, d], bf16, kind="ExternalInput")  # [d, e]
    wkT = nc.dram_tensor("wkT", [d, d], bf16, kind="ExternalInput")
    wvT = nc.dram_tensor("wvT", [d, d], bf16, kind="ExternalInput")
    bq = nc.dram_tensor("bq", [d], f32, kind="ExternalInput")
    bk = nc.dram_tensor("bk", [d], f32, kind="ExternalInput")
    bv = nc.dram_tensor("bv", [d], f32, kind="ExternalInput")
    qres = nc.dram_tensor("qres", [s, d], f32, kind="ExternalInput")
    attn_o = nc.dram_tensor("attn", [s, d], f32, kind="ExternalOutput")
    attnw_o = nc.dram_tensor("attn_w", [s, d], f32, kind="ExternalOutput")

    qT_r = qT[:].rearrange("(dt p) s -> p dt s", p=P)
    kT_r = kT[:].rearrange("(dt p) s -> p dt s", p=P)
    vT_r = vT[:].rearrange("(dt p) s -> p dt s", p=P)
    w_rs = [
        w[:].rearrange("(dt p) e -> p dt e", p=P) for w in (wqT, wkT, wvT)
    ]
    bq_r = bq[:].rearrange("(t p) -> p t", p=P)
    bk_r = bk[:].rearrange("(t p) -> p t", p=P)
    qres_r = qres[:].rearrange("(st p) d -> p st d", p=P)
    attn_r = attn_o[:].rearrange("(st p) d -> p st d", p=P)
    attnw_r = attnw_o[:].rearrange("(st p) d -> p st d", p=P)

    with tile.TileContext(nc) as tc:
        with (
            tc.tile_pool(name="consts", bufs=1) as consts,
            tc.tile_pool(name="big", bufs=1) as big,
            tc.tile_pool(name="io", bufs=2) as io,
            tc.tile_pool(name="small", bufs=4) as small,
            tc.tile_pool(name="psum", bufs=4, space="PSUM") as psum,
            tc.tile_pool(name="psum1", bufs=1, space="PSUM") as psum1,
        ):
            # ---- constants ----
            bq_t = consts.tile([P, ET], f32)
            nc.sync.dma_start(out=bq_t[:], in_=bq_r)
            bk_t = consts.tile([P, ET], f32)
            nc.sync.dma_start(out=bk_t[:], in_=bk_r)
            # bv broadcast across partitions (varies along free axis e)
            bv_bc = consts.tile([P, d], f32)
            bv_ap = bv[:]
            bv_bcast_src = bass.AP(
                tensor=bv_ap.tensor, offset=bv_ap.offset, ap=[[0, P], [1, d]]
            )
            nc.sync.dma_start(out=bv_bc[:], in_=bv_bcast_src)
            ones_col = consts.tile([P, 1], bf16)   # lhsT for column sums (K=P, M=1)
            nc.vector.memset(ones_col[:], 1.0)
            ones_row = consts.tile([1, P], f32)    # lhsT for 1/Z broadcast (K=1, M=P)
            nc.vector.memset(ones_row[:], 1.0)
            rz_all = consts.tile([P, ST], f32)     # per-k-row 1/Z of softmax #1
            rz2 = consts.tile([1, d], f32)         # 1/colsum of softmax #2

            # ---- resident tensors (tag-slot reuse, see module docstring) ----
            wall = big.tile([P, 3, DT, d], bf16, tag="W")   # wq|wk|wv
            for wi in range(3):
                nc.sync.dma_start(out=wall[:, wi], in_=w_rs[wi])
            qp = big.tile([P, ET, s], bf16, tag="A")        # qpT: [e, q]
            kp = big.tile([P, ET, s], bf16, tag="B")        # kpT: [e, k]
            vp = big.tile([P, ST, d], bf16, tag="V")        # natural [s, e]

            # ---- Phase 1a: qpT / kpT projections ----
            for src_r, wi, b_t, dst in (
                (qT_r, 0, bq_t, qp),
                (kT_r, 1, bk_t, kp),
            ):
                for qc in range(QC):
                    xt = io.tile([P, DT, NFQ], bf16, tag="xin")
                    nc.sync.dma_start(
                        out=xt[:], in_=src_r[:, :, qc * NFQ:(qc + 1) * NFQ]
                    )
                    for et in range(ET):
                        ps = psum.tile([P, NFQ], f32, tag="ps")
                        for dt_ in range(DT):
                            nc.tensor.matmul(
                                ps[:],
                                wall[:, wi, dt_, et * P:(et + 1) * P],
                                xt[:, dt_, :],
                                start=(dt_ == 0),
                                stop=(dt_ == DT - 1),
                            )
                        # bias add (per-partition) + bf16 cast on ScalarE
                        nc.scalar.activation(
                            out=dst[:, et, qc * NFQ:(qc + 1) * NFQ],
                            in_=ps[:],
                            func=mybir.ActivationFunctionType.Identity,
                            bias=b_t[:, et:et + 1],
                        )

            # ---- Phase 1b: vp projection (natural layout) ----
            for sc in range(SC):
                vt = io.tile([P, DT, SCW], bf16, tag="xin")
                nc.sync.dma_start(
                    out=vt[:], in_=vT_r[:, :, sc * SCW:(sc + 1) * SCW]
                )
                for sti in range(SCW // P):
                    st = sc * (SCW // P) + sti
                    for ec in range(EC):
                        ps = psum.tile([P, NFD], f32, tag="ps")
                        for dt_ in range(DT):
                            nc.tensor.matmul(
                                ps[:],
                                vt[:, dt_, sti * P:(sti + 1) * P],
                                wall[:, 2, dt_, ec * NFD:(ec + 1) * NFD],
                                start=(dt_ == 0),
                                stop=(dt_ == DT - 1),
                            )
                        nc.vector.tensor_add(
                            out=vp[:, st, ec * NFD:(ec + 1) * NFD],
                            in0=ps[:],
                            in1=bv_bc[:, ec * NFD:(ec + 1) * NFD],
                        )

            # ---- Phase 2: scoresT -> softmax over q -> probs ----
            # probs reuses the weights' slot (tag W).
            # No max-subtraction: |scores/d| < ~0.3 by construction.
            probs = big.tile([P, ST, s], bf16, tag="W")     # [k, q] per k-tile
            for kt in range(ST):
                partials = small.tile([P, QC], f32, tag="partials")
                for qc in range(QC):
                    ps = psum.tile([P, NFQ], f32, tag="ps")
                    for et in range(ET):
                        nc.tensor.matmul(
                            ps[:],
                            kp[:, et, kt * P:(kt + 1) * P],
                            qp[:, et, qc * NFQ:(qc + 1) * NFQ],
                            start=(et == 0),
                            stop=(et == ET - 1),
                        )
                    nc.scalar.activation(
                        out=probs[:, kt, qc * NFQ:(qc + 1) * NFQ],
                        in_=ps[:],
                        func=mybir.ActivationFunctionType.Exp,
                        scale=1.0 / d,
                        accum_out=partials[:, qc:qc + 1],
                    )
                zsum = small.tile([P, 1], f32, tag="zsum")
                nc.vector.reduce_sum(
                    out=zsum[:], in_=partials[:], axis=mybir.AxisListType.X
                )
                nc.vector.reciprocal(out=rz_all[:, kt:kt + 1], in_=zsum[:])
                # fold 1/Z[k] into vp's k-rows (cheaper than scaling probs)
                nc.vector.tensor_scalar_mul(
                    out=vp[:, kt, :],
                    in0=vp[:, kt, :],
                    scalar1=rz_all[:, kt:kt + 1],
                )

            # ---- Phase 3: attn = probsT.T @ vp ; residual; exp(attn) ----
            # expb reuses qp's slot (tag A).
            expb = big.tile([P, ST, d], bf16, tag="A")      # exp(attn), bf16
            cs_ps = psum1.tile([1, d], f32, tag="cs")       # colsums of exp(attn)
            for st in range(ST):
                qres_t = io.tile([P, d], f32, tag="xin")
                nc.sync.dma_start(out=qres_t[:], in_=qres_r[:, st, :])
                for ec in range(EC):
                    ps = psum.tile([P, NFD], f32, tag="ps")
                    for kt in range(ST):
                        nc.tensor.matmul(
                            ps[:],
                            probs[:, kt, st * P:(st + 1) * P],
                            vp[:, kt, ec * NFD:(ec + 1) * NFD],
                            start=(kt == 0),
                            stop=(kt == ST - 1),
                        )
                    ao = io.tile([P, NFD], f32, tag="ao")
                    nc.vector.tensor_add(
                        out=ao[:],
                        in0=ps[:],
                        in1=qres_t[:, ec * NFD:(ec + 1) * NFD],
                    )
                    nc.sync.dma_start(
                        out=attn_r[:, st, ec * NFD:(ec + 1) * NFD], in_=ao[:]
                    )
                    nc.scalar.activation(
                        out=expb[:, st, ec * NFD:(ec + 1) * NFD],
                        in_=ps[:],
                        func=mybir.ActivationFunctionType.Exp,
                    )
                    nc.tensor.matmul(
                        cs_ps[:, ec * NFD:(ec + 1) * NFD],
                        ones_col[:],
                        expb[:, st, ec * NFD:(ec + 1) * NFD],
                        start=(st == 0),
                        stop=(st == ST - 1),
                    )

            # ---- Phase 3.5: 1/colsum, broadcast across partitions ----
            nc.vector.reciprocal(out=rz2[:], in_=cs_ps[:])
            rzb_ps = psum1.tile([P, d], f32, tag="cs")      # reuses cs_ps bank
            for ec in range(EC):
                nc.tensor.matmul(
                    rzb_ps[:, ec * NFD:(ec + 1) * NFD],
                    ones_row[:],
                    rz2[:, ec * NFD:(ec + 1) * NFD],
                    start=True,
                    stop=True,
                )

            # ---- Phase 4: attn_w = exp(attn) * (1/colsum) ----
            for st in range(ST):
                aw = io.tile([P, d], f32, tag="xin")
                nc.vector.tensor_mul(out=aw[:], in0=expb[:, st, :], in1=rzb_ps[:])
                nc.sync.dma_start(out=attnw_r[:, st, :], in_=aw[:])

    return nc


def _host_prep(q, k, v, Wq, bq, Wk, bk, Wv, bv):
    """Shard over batch and pre-transpose/cast on host."""
    bf16 = ml_dtypes.bfloat16
    q = np.asarray(q, dtype=np.float32)
    k = np.asarray(k, dtype=np.float32)
    v = np.asarray(v, dtype=np.float32)
    wqT = np.asarray(Wq, dtype=np.float32).T.astype(bf16)  # [d, e]
    wkT = np.asarray(Wk, dtype=np.float32).T.astype(bf16)
    wvT = np.asarray(Wv, dtype=np.float32).T.astype(bf16)
    bq = np.ascontiguousarray(np.asarray(bq, dtype=np.float32))
    bk = np.ascontiguousarray(np.asarray(bk, dtype=np.float32))
    bv = np.ascontiguousarray(np.asarray(bv, dtype=np.float32))

    in_maps = []
    for i in range(B):
        in_maps.append(
            {
                "qT": q[i].T.astype(bf16),
                "kT": k[i].T.astype(bf16),
                "vT": v[i].T.astype(bf16),
                "wqT": wqT,
                "wkT": wkT,
                "wvT": wvT,
                "bq": bq,
                "bk": bk,
                "bv": bv,
                "qres": np.ascontiguousarray(q[i]),
            }
        )
    return in_maps


_CACHED_NC = None


def kernel(q, k, v, Wq, bq, Wk, bk, Wv, bv):
    global _CACHED_NC
    from concourse import bass_utils

    in_maps = _host_prep(q, k, v, Wq, bq, Wk, bk, Wv, bv)
    if _CACHED_NC is None:
        _CACHED_NC = build_nc()
        _CACHED_NC.finalize()  # bacc passes (reg alloc, wait splitting)
    res = bass_utils.run_bass_kernel_spmd(
        _CACHED_NC, in_maps, core_ids=list(range(B))
    )
    attn = np.stack([np.asarray(res.results[i]["attn"]) for i in range(B)])
    attn_w = np.stack([np.asarray(res.results[i]["attn_w"]) for i in range(B)])
    return attn.astype(np.float32), attn_w.astype(np.float32)


# revision 18
# speedup vs baseline: 1.0218x; 1.0218x over previous
"""Trainium2 Bass kernel for nn_MultiHeadAttention_66872640799208.

Math (per batch element b, S=2048, D=1024):
    qp = q @ Wq.T + bq ; kp = k @ Wk.T + bk ; vp = v @ Wv.T + bv
    scores = qp @ kp.T / D
    probs  = softmax(scores, axis=q)          # over the QUERY axis
    attn   = probs @ vp
    attn_w = softmax(attn, axis=q)            # over the sequence axis
    out    = (attn + q, attn_w)

Sharding: data-parallel over batch B=8 -> one batch element per NeuronCore,
no collectives. Host pre-transposes inputs to bf16 so that every matmul
contracts over the partition axis with no on-chip transposes:
  - qT/kT/vT [D, S] feed the projections (contraction over d),
  - qpT/kpT are produced in [e, s] layout so scoresT = kpT.T @ qpT has the
    softmax axis (q) on the free dimension,
  - vp is produced in natural [s, e] layout as lhsT/rhs of the attn matmul,
  - the probs 1/Z normalization (Z indexed by k) is folded into vp's rows
    (k is vp's partition axis) instead of scaling the much larger probs.
The second softmax (over q, the partition axis of attn) uses a ones-vector
PE matmul for the column sums and a K=1 PE matmul to broadcast 1/Z.

SBUF (192KB/partition budget) is managed with tag-slot reuse in one pool:
  tag A: qp (P1-P2) -> expb=exp(attn) (P3-P4)        32KB
  tag B: kp (P1-P2)                                  32KB
  tag W: weights (P1) -> probs (P2-P3)               64KB (max)
  tag V: vp (P1-P3)                                  32KB
"""

import sys

if "/opt/trn_rl_repo" not in sys.path:
    sys.path.insert(0, "/opt/trn_rl_repo")

import numpy as np
import ml_dtypes

B = 8
S = 2048
D = 1024
P = 128


def build_nc(s=S, d=D):
    """Build the single-core Bass program (SPMD: identical on all cores)."""
    import concourse.bass as bass
    import concourse.tile as tile
    from concourse import bacc, mybir

    bf16 = mybir.dt.bfloat16
    f32 = mybir.dt.float32

    DT = d // P          # contraction tiles for projections
    ET = d // P          # e (feature) tiles
    ST = s // P          # sequence tiles
    NFQ = min(512, s)    # matmul moving free-dim over q
    QC = s // NFQ        # q chunks
    NFD = min(512, d)    # matmul moving free-dim over d/e
    EC = d // NFD        # e chunks
    SCW = min(512, s)    # vT stream chunk width (in s)
    SC = s // SCW

    nc = bacc.Bacc("TRN2")

    qT = nc.dram_tensor("qT", [d, s], bf16, kind="ExternalInput")
    kT = nc.dram_tensor("kT", [d, s], bf16, kind="ExternalInput")
    vT = nc.dram_tensor("vT", [d, s], bf16, kind="ExternalInput")
    wqT = nc.dram_tensor("wqT", [d, d], bf16, kind="ExternalInput")  # [d, e]
    wkT = nc.dram_tensor("wkT", [d, d], bf16, kind="ExternalInput")
    wvT = nc.dram_tensor("wvT", [d, d], bf16, kind="ExternalInput")
    bq = nc.dram_tensor("bq", [d], f32, kind="ExternalInput")
    bk = nc.dram_tensor("bk", [d], f32, kind="ExternalInput")
    bv = nc.dram_tensor("bv", [d], f32, kind="ExternalInput")
    qres = nc.dram_tensor("qres", [s, d], f32, kind="ExternalInput")
    attn_o = nc.dram_tensor("attn", [s, d], f32, kind="ExternalOutput")
    attnw_o = nc.dram_tensor("attn_w", [s, d], f32, kind="ExternalOutput")

    qT_r = qT[:].rearrange("(dt p) s -> p dt s", p=P)
    kT_r = kT[:].rearrange("(dt p) s -> p dt s", p=P)
    vT_r = vT[:].rearrange("(dt p) s -> p dt s", p=P)
    w_rs = [
        w[:].rearrange("(dt p) e -> p dt e", p=P) for w in (wqT, wkT, wvT)
    ]
    bq_r = bq[:].rearrange("(t p) -> p t", p=P)
    bk_r = bk[:].rearrange("(t p) -> p t", p=P)
    qres_r = qres[:].rearrange("(st p) d -> p st d", p=P)
    attn_r = attn_o[:].rearrange("(st p) d -> p st d", p=P)
    attnw_r = attnw_o[:].rearrange("(st p) d -> p st d", p=P)

    with tile.TileContext(nc) as tc:
        with (
            tc.tile_pool(name="consts", bufs=1) as consts,
            tc.tile_pool(name="big", bufs=1) as big,
            tc.tile_pool(name="io", bufs=2) as io,
            tc.tile_pool(name="small", bufs=4) as small,
            tc.tile_pool(name="psum", bufs=4, space="PSUM") as psum,
            tc.tile_pool(name="psum1", bufs=1, space="PSUM") as psum1,
        ):
            # ---- constants (DMA order matters: wq + first q-chunk first so
            # the PE can start projecting ~8us in; the rest stream behind) ----
            wall = big.tile([P, 3, DT, d], bf16, tag="W")   # wq|wk|wv
            nc.sync.dma_start(out=wall[:, 0], in_=w_rs[0])
            bq_t = consts.tile([P, ET], f32)
            nc.sync.dma_start(out=bq_t[:], in_=bq_r)
            bk_t = consts.tile([P, ET], f32)
            bv_bc = consts.tile([P, d], f32)
            ones_col = consts.tile([P, 1], bf16)   # lhsT for column sums (K=P, M=1)
            nc.vector.memset(ones_col[:], 1.0)
            rz_all = consts.tile([P, ST], f32)     # per-k-row 1/Z of softmax #1
            rz2 = consts.tile([1, d], f32)         # 1/colsum of softmax #2
            ones_row = consts.tile([1, P], f32)    # lhsT for 1/Z broadcast (K=1)
            nc.vector.memset(ones_row[:], 1.0)

            qp = big.tile([P, ET, s], bf16, tag="A")        # qpT: [e, q]
            kp = big.tile([P, ET, s], bf16, tag="B")        # kpT: [e, k]
            vp = big.tile([P, ST, d], bf16, tag="V")        # natural [s, e]

            # ---- Phase 1a: qpT / kpT projections ----
            for src_r, wi, b_t, dst in (
                (qT_r, 0, bq_t, qp),
                (kT_r, 1, bk_t, kp),
            ):
                for qc in range(QC):
                    xt = io.tile([P, DT, NFQ], bf16, tag="xin")
                    nc.sync.dma_start(
                        out=xt[:], in_=src_r[:, :, qc * NFQ:(qc + 1) * NFQ]
                    )
                    if wi == 0 and qc == 0:
                        # stream the remaining weights behind the first chunk
                        nc.sync.dma_start(out=wall[:, 1], in_=w_rs[1])
                        nc.sync.dma_start(out=wall[:, 2], in_=w_rs[2])
                        nc.sync.dma_start(out=bk_t[:], in_=bk_r)
                        bv_ap = bv[:]
                        nc.sync.dma_start(
                            out=bv_bc[:],
                            in_=bass.AP(
                                tensor=bv_ap.tensor,
                                offset=bv_ap.offset,
                                ap=[[0, P], [1, d]],
                            ),
                        )
                    for et in range(ET):
                        ps = psum.tile([P, NFQ], f32, tag="ps")
                        for dt_ in range(DT):
                            nc.tensor.matmul(
                                ps[:],
                                wall[:, wi, dt_, et * P:(et + 1) * P],
                                xt[:, dt_, :],
                                start=(dt_ == 0),
                                stop=(dt_ == DT - 1),
                            )
                        # bias add (per-partition) + bf16 cast on ScalarE
                        nc.scalar.activation(
                            out=dst[:, et, qc * NFQ:(qc + 1) * NFQ],
                            in_=ps[:],
                            func=mybir.ActivationFunctionType.Identity,
                            bias=b_t[:, et:et + 1],
                        )

            # ---- Phase 1b: vp projection (natural layout) ----
            for sc in range(SC):
                vt = io.tile([P, DT, SCW], bf16, tag="xin")
                nc.sync.dma_start(
                    out=vt[:], in_=vT_r[:, :, sc * SCW:(sc + 1) * SCW]
                )
                for sti in range(SCW // P):
                    st = sc * (SCW // P) + sti
                    for ec in range(EC):
                        ps = psum.tile([P, NFD], f32, tag="ps")
                        for dt_ in range(DT):
                            nc.tensor.matmul(
                                ps[:],
                                vt[:, dt_, sti * P:(sti + 1) * P],
                                wall[:, 2, dt_, ec * NFD:(ec + 1) * NFD],
                                start=(dt_ == 0),
                                stop=(dt_ == DT - 1),
                            )
                        nc.vector.tensor_add(
                            out=vp[:, st, ec * NFD:(ec + 1) * NFD],
                            in0=ps[:],
                            in1=bv_bc[:, ec * NFD:(ec + 1) * NFD],
                        )

            # ---- Phase 2: scoresT -> softmax over q -> probs ----
            # probs reuses the weights' slot (tag W).
            # No max-subtraction: |scores/d| < ~0.3 by construction.
            probs = big.tile([P, ST, s], bf16, tag="W")     # [k, q] per k-tile
            for kt in range(ST):
                partials = small.tile([P, QC], f32, tag="partials")
                for qc in range(QC):
                    ps = psum.tile([P, NFQ], f32, tag="ps")
                    for et in range(ET):
                        nc.tensor.matmul(
                            ps[:],
                            kp[:, et, kt * P:(kt + 1) * P],
                            qp[:, et, qc * NFQ:(qc + 1) * NFQ],
                            start=(et == 0),
                            stop=(et == ET - 1),
                        )
                    nc.scalar.activation(
                        out=probs[:, kt, qc * NFQ:(qc + 1) * NFQ],
                        in_=ps[:],
                        func=mybir.ActivationFunctionType.Exp,
                        scale=1.0 / d,
                        accum_out=partials[:, qc:qc + 1],
                    )
                zsum = small.tile([P, 1], f32, tag="zsum")
                nc.vector.reduce_sum(
                    out=zsum[:], in_=partials[:], axis=mybir.AxisListType.X
                )
                nc.vector.reciprocal(out=rz_all[:, kt:kt + 1], in_=zsum[:])
                # fold 1/Z[k] into vp's k-rows (cheaper than scaling probs)
                nc.vector.tensor_scalar_mul(
                    out=vp[:, kt, :],
                    in0=vp[:, kt, :],
                    scalar1=rz_all[:, kt:kt + 1],
                )

            # ---- Phase 3: attn = probsT.T @ vp ; residual; exp(attn) ----
            # expb reuses qp's slot (tag A).
            expb = big.tile([P, ST, d], bf16, tag="A")      # exp(attn), bf16
            cs_ps = psum1.tile([1, d], f32, tag="cs")       # colsums of exp(attn)
            for st in range(ST):
                qres_t = io.tile([P, d], f32, tag="xin")
                nc.sync.dma_start(out=qres_t[:], in_=qres_r[:, st, :])
                for ec in range(EC):
                    ps = psum.tile([P, NFD], f32, tag="ps")
                    for kt in range(ST):
                        nc.tensor.matmul(
                            ps[:],
                            probs[:, kt, st * P:(st + 1) * P],
                            vp[:, kt, ec * NFD:(ec + 1) * NFD],
                            start=(kt == 0),
                            stop=(kt == ST - 1),
                        )
                    ao = io.tile([P, NFD], f32, tag="ao")
                    nc.vector.tensor_add(
                        out=ao[:],
                        in0=ps[:],
                        in1=qres_t[:, ec * NFD:(ec + 1) * NFD],
                    )
                    nc.sync.dma_start(
                        out=attn_r[:, st, ec * NFD:(ec + 1) * NFD], in_=ao[:]
                    )
                    nc.scalar.activation(
                        out=expb[:, st, ec * NFD:(ec + 1) * NFD],
                        in_=ps[:],
                        func=mybir.ActivationFunctionType.Exp,
                    )
                    nc.tensor.matmul(
                        cs_ps[:, ec * NFD:(ec + 1) * NFD],
                        ones_col[:],
                        expb[:, st, ec * NFD:(ec + 1) * NFD],
                        start=(st == 0),
                        stop=(st == ST - 1),
                    )

            # ---- Phase 3.5: 1/colsum, broadcast across partitions ----
            nc.vector.reciprocal(out=rz2[:], in_=cs_ps[:])
            rzb = psum1.tile([P, d], f32, tag="cs")         # reuses cs_ps bank
            for ec in range(EC):
                nc.tensor.matmul(
                    rzb[:, ec * NFD:(ec + 1) * NFD],
                    ones_row[:],
                    rz2[:, ec * NFD:(ec + 1) * NFD],
                    start=True,
                    stop=True,
                )

            # ---- Phase 4: attn_w = exp(attn) * (1/colsum) ----
            for st in range(ST):
                aw = io.tile([P, d], f32, tag="xin")
                nc.vector.tensor_mul(out=aw[:], in0=expb[:, st, :], in1=rzb[:])
                nc.sync.dma_start(out=attnw_r[:, st, :], in_=aw[:])

    return nc


def _host_prep(q, k, v, Wq, bq, Wk, bk, Wv, bv):
    """Shard over batch and pre-transpose/cast on host."""
    bf16 = ml_dtypes.bfloat16
    q = np.asarray(q, dtype=np.float32)
    k = np.asarray(k, dtype=np.float32)
    v = np.asarray(v, dtype=np.float32)
    wqT = np.asarray(Wq, dtype=np.float32).T.astype(bf16)  # [d, e]
    wkT = np.asarray(Wk, dtype=np.float32).T.astype(bf16)
    wvT = np.asarray(Wv, dtype=np.float32).T.astype(bf16)
    bq = np.ascontiguousarray(np.asarray(bq, dtype=np.float32))
    bk = np.ascontiguousarray(np.asarray(bk, dtype=np.float32))
    bv = np.ascontiguousarray(np.asarray(bv, dtype=np.float32))

    in_maps = []
    for i in range(B):
        in_maps.append(
            {
                "qT": q[i].T.astype(bf16),
                "kT": k[i].T.astype(bf16),
                "vT": v[i].T.astype(bf16),
                "wqT": wqT,
                "wkT": wkT,
                "wvT": wvT,
                "bq": bq,
                "bk": bk,
                "bv": bv,
                "qres": np.ascontiguousarray(q[i]),
            }
        )
    return in_maps


_CACHED_NC = None


def kernel(q, k, v, Wq, bq, Wk, bk, Wv, bv):
    global _CACHED_NC
    from concourse import bass_utils

    in_maps = _host_prep(q, k, v, Wq, bq, Wk, bk, Wv, bv)
    if _CACHED_NC is None:
        _CACHED_NC = build_nc()
        _CACHED_NC.finalize()  # bacc passes (reg alloc, wait splitting)
    res = bass_utils.run_bass_kernel_spmd(
        _CACHED_NC, in_maps, core_ids=list(range(B))
    )
    attn = np.stack([np.asarray(res.results[i]["attn"]) for i in range(B)])
    attn_w = np.stack([np.asarray(res.results[i]["attn_w"]) for i in range(B)])
    return attn.astype(np.float32), attn_w.astype(np.float32)
